# revision 11
# baseline (speedup 1.0000x reference)
"""Trainium2 Bass kernel for LLFullObjectCondensation loss (N=80000, K=512, C=2).

Object-aligned sharding (8 NeuronCores), v4:
  - Core c owns 64 objects; hits of those objects are assigned to core c.
  - Attraction expanded polynomially into segment sums.
  - The repulsion block gets 64 EXTRA columns duplicating the core's own
    object window (from local alphas, no collective needed) so the
    own-alpha hinge values sit at a static column slice; the rep
    self-correction is extracted per tile on the otherwise-idle GpSimd
    engine and feeds the segment matmul in the same loop.
  - L_rep linearized over per-core rep row-sums: per-object coefficients
    are AllGathered and each core contributes a local dot product.
  - Collectives: barrier + AllGather(768B alphas) + AllGather(272B
    coef+extras) + AllGather(32B partials).
"""
import sys
import numpy as np

for _p in ("/opt/trn_rl_repo", "/root/.axon_site/_ro/trn_rl_repo"):
    if _p not in sys.path:
        sys.path.append(_p)

N = 80000
K = 512
NCORES = 8
OW = K // NCORES         # 64 objects per core
P = 128
EPS = 1e-9
SQ_BIAS = 1e-6           # matches reference's d2+1e-6 inside the hinge sqrt

_CACHE = {}


def _build(T):
    import concourse.bass as bass
    import concourse.bacc as bacc
    import concourse.mybir as mybir
    import concourse.tile as tile
    from concourse import masks

    f32 = mybir.dt.float32
    i32 = mybir.dt.int32
    bf16 = mybir.dt.bfloat16
    AF = mybir.ActivationFunctionType
    OP = mybir.AluOpType
    AX = mybir.AxisListType

    nc = bacc.Bacc("TRN2", target_bir_lowering=False, debug=False,
                   num_devices=NCORES)

    di = {}
    def din(name, shape):
        di[name] = nc.dram_tensor(name, shape, f32, kind="ExternalInput")
        return di[name]

    din("beta_r", [P, T])
    din("cc", [P, T, 2])
    din("pE", [P, T])
    din("ppos", [P, T, 2])
    din("ptime", [P, T])
    din("pid", [P, T, 6])
    din("tE", [P, T])
    din("tpos", [P, T, 2])
    din("ttime", [P, T])
    din("tidx", [P, T])
    din("valid", [P, T])
    out_d = nc.dram_tensor("out", [1, 1], f32, kind="ExternalOutput")
    GRP = [list(range(NCORES))]

    with tile.TileContext(nc) as tc:
        with (
            tc.tile_pool(name="const", bufs=1) as cpool,
            tc.tile_pool(name="io", bufs=1) as io,
            tc.tile_pool(name="dram", bufs=1, space="DRAM") as dram,
            tc.tile_pool(name="acc", bufs=1, space="PSUM") as accp,
        ):
            # transpose pool, closed manually before the main loop so its
            # PSUM banks go to the d2 pipeline
            psTc = tc.tile_pool(name="psT", bufs=2, space="PSUM")
            psT = psTc.__enter__()
            # ---------- constants ----------
            ident = cpool.tile([P, P], f32)
            masks.make_identity(nc, ident[:])
            iotaWI = cpool.tile([P, OW], i32)
            nc.gpsimd.iota(iotaWI[:], pattern=[[1, OW]], base=0,
                           channel_multiplier=0)
            iotaW = cpool.tile([P, OW], f32)
            nc.vector.tensor_copy(iotaW[:], iotaWI[:])
            onescol = cpool.tile([P, 1], f32)
            nc.vector.memset(onescol[:], 1.0)
            onesrow = cpool.tile([1, P], f32)
            nc.vector.memset(onesrow[:], 1.0)

            _cb = {}
            def cbias(val):
                if val not in _cb:
                    ct = cpool.tile([P, 1], f32, name=f"cb{len(_cb)}")
                    nc.vector.memset(ct[:], val)
                    _cb[val] = ct
                return _cb[val][:]

            # ---------- early barrier: absorb NEFF launch skew ----------
            bar_in = dram.tile([1, 8], f32, name="bar_in")
            bar_out = dram.tile([1, 64], f32, name="bar_out",
                                addr_space="Shared")
            barsb = cpool.tile([1, 8], f32)
            nc.vector.memset(barsb[:], 0.0)
            nc.sync.dma_start(bar_in[0:1, :], barsb[:])
            nc.gpsimd.collective_compute(
                "AllGather", mybir.AluOpType.bypass,
                replica_groups=GRP,
                ins=[bar_in[:]], outs=[bar_out[:]],
            )

            # ---------- load inputs ----------
            sb = {}
            for name, h in di.items():
                t_sb = io.tile(list(h.shape), f32, name=f"sb_{name}")
                nc.sync.dma_start(t_sb[:], h.ap())
                sb[name] = t_sb

            V = nc.vector
            SC = nc.scalar
            GP = nc.gpsimd

            def wtile(name, shape=None, dtype=None):
                return io.tile(shape or [P, T], dtype or f32, name=name)
            u8 = mybir.dt.uint8

            # ---------- critical chain: beta -> membership + max ----------
            beta = wtile("beta")
            V.tensor_scalar(beta[:], sb["beta_r"][:], 1e-6, 1.0 - 1e-6,
                            OP.max, OP.min)

            # q chain pieces needed for the early count accumulation
            rbeta = wtile("rbeta")
            V.reciprocal(rbeta[:], beta[:])
            betap1 = wtile("betap1")
            SC.activation(betap1[:], beta[:], AF.Identity, bias=cbias(1.0))
            onem = wtile("onem")
            SC.activation(onem[:], beta[:], AF.Identity, bias=cbias(1.0),
                          scale=-1.0)
            recm = wtile("recm")
            V.reciprocal(recm[:], onem[:])
            ratio = wtile("ratio")
            V.tensor_tensor(ratio[:], betap1[:], recm[:], OP.mult)
            lnr = wtile("lnr")
            SC.activation(lnr[:], ratio[:], AF.Ln)
            halfln = wtile("halfln")
            SC.activation(halfln[:], lnr[:], AF.Copy, scale=0.5)
            q = wtile("q")
            V.tensor_tensor(q[:], halfln[:], halfln[:], OP.mult)
            V.scalar_tensor_tensor(q[:], q[:], 0.1, sb["valid"][:],
                                   OP.add, OP.mult)
            qrb = wtile("qrb")
            V.tensor_tensor(qrb[:], q[:], rbeta[:], OP.mult)
            cnt2 = wtile("cnt2", [P, T, 2])
            V.tensor_tensor(cnt2[:, :, 0], sb["valid"][:], rbeta[:], OP.mult)
            V.tensor_copy(cnt2[:, :, 1], qrb[:])

            bmBs = io.tile([P, T, OW], f32, name="bmBs")
            cntsb = io.tile([OW, 2], f32, name="cntsb")
            with tc.tile_pool(name="cntpp", bufs=1, space="PSUM") as cntpp:
                cntP = cntpp.tile([2, OW], f32, name="cntP")
                V.memset(cntP[:], 0.0)
                for t in range(T):
                    V.tensor_scalar(bmBs[:, t, :], iotaW[:],
                                    sb["tidx"][:, t:t + 1], beta[:, t:t + 1],
                                    OP.is_equal, OP.mult)
                    nc.tensor.matmul(cntP[:], cnt2[:, t, :], bmBs[:, t, :],
                                     start=False, stop=(t == T - 1),
                                     skip_group_check=True)
                cntT = io.tile([2, OW], f32, name="cntT")
                SC.activation(cntT[:], cntP[:], AF.Copy)
                tpc = psT.tile([P, P], f32, name="tpc", tag="tpose")
                nc.tensor.transpose(tpc[0:OW, 0:2], cntT[:], ident[0:2, 0:2])
                SC.activation(cntsb[:], tpc[0:OW, 0:2], AF.Copy)

            # pairwise max-reduction tree over tiles (breaks the serial
            # running-max dependency chain)
            mtA = io.tile([P, (T + 1) // 2, OW], f32, name="mtA")
            mtB = io.tile([P, (T + 3) // 4, OW], f32, name="mtB")
            h = T // 2
            V.tensor_tensor(mtA[:, 0:h, :], bmBs[:, 0:2 * h:2, :],
                            bmBs[:, 1:2 * h:2, :], OP.max)
            if T % 2:
                V.tensor_copy(mtA[:, h:h + 1, :], bmBs[:, T - 1:T, :])
                h += 1
            cur, nxt = mtA, mtB
            while h > 1:
                h2 = h // 2
                V.tensor_tensor(nxt[:, 0:h2, :], cur[:, 0:2 * h2:2, :],
                                cur[:, 1:2 * h2:2, :], OP.max)
                if h % 2:
                    V.tensor_copy(nxt[:, h2:h2 + 1, :],
                                  cur[:, h - 1:h, :])
                    h2 += 1
                cur, nxt = nxt, cur
                h = h2
            runmax = cur[:, 0, :]

            Bmax = io.tile([OW, 1], f32, name="Bmax")
            tpm = psT.tile([P, P], f32, name="tpm", tag="tpose")
            nc.tensor.transpose(tpm[0:OW, :], runmax, ident[:])
            V.reduce_max(Bmax[:], tpm[0:OW, :], axis=AX.X)
            BmaxF = io.tile([1, OW], f32, name="BmaxF")
            tpf = psT.tile([P, P], f32, name="tpf", tag="tpose")
            nc.tensor.transpose(tpf[0:1, 0:OW], Bmax[:], ident[0:OW, 0:OW])
            SC.activation(BmaxF[:], tpf[0:1, 0:OW], AF.Copy)
            BlocB = io.tile([P, OW], f32, name="BlocB")
            with tc.tile_pool(name="bcp", bufs=1, space="PSUM") as bcp:
                bps = bcp.tile([P, OW], f32, name="bps")
                nc.tensor.matmul(bps[:], onesrow[:], BmaxF[:],
                                 start=True, stop=True)
                SC.activation(BlocB[:], bps[:], AF.Copy)

            # ---------- remaining q-derived quantities ----------
            qrb2 = wtile("qrb2")
            V.tensor_tensor(qrb2[:], qrb[:], rbeta[:], OP.mult)
            qbf = wtile("qbf", dtype=bf16)
            SC.activation(qbf[:], q[:], AF.Copy)

            # d2-matmul lhsT quantities + transposes
            prep4 = wtile("prep4", [P, T, 4])
            SC.activation(prep4[:, :, 0:2], sb["cc"][:], AF.Copy, scale=-2.0)
            V.memset(prep4[:, :, 2], 1.0)
            ccsq = wtile("ccsq", [P, T, 2])
            V.tensor_tensor(ccsq[:], sb["cc"][:], sb["cc"][:], OP.mult)
            V.tensor_tensor(prep4[:, :, 3], ccsq[:, :, 0], ccsq[:, :, 1],
                            OP.add)
            lhsT4 = io.tile([4, T, P], bf16, name="lhsT4")
            for r in range(4):
                tp = psT.tile([P, P], f32, name="tpose3", tag="tpose")
                nc.tensor.transpose(tp[0:T, :], prep4[:, :, r], ident[:])
                stage = io.tile([T, P], bf16, name=f"tstage{r}")
                SC.activation(stage[:], tp[0:T, :], AF.Copy)
                nc.sync.dma_start(lhsT4[r:r + 1, :, :], stage[:])

            # selection rhs: [x0, x1, q]
            sel3 = wtile("sel3", [P, T, 3])
            SC.activation(sel3[:, :, 0:2], sb["cc"][:], AF.Copy)
            V.tensor_copy(sel3[:, :, 2], q[:])

            # ---------- loopB: alpha selection (core-local) ----------
            selsb = io.tile([OW, 3], f32, name="selsb")
            with (
                tc.tile_pool(name="selpp", bufs=1, space="PSUM") as selpp,
                tc.tile_pool(name="isp", bufs=3) as isp,
            ):
                selP = selpp.tile([3, OW], f32, name="selP")
                V.memset(selP[:], 0.0)
                for t in range(T):
                    Isel = isp.tile([P, OW], f32, name="Isel")
                    V.tensor_tensor(Isel[:], bmBs[:, t, :], BlocB[:],
                                    OP.is_equal)
                    nc.tensor.matmul(selP[:], sel3[:, t, :], Isel[:],
                                     start=False, stop=(t == T - 1),
                                     skip_group_check=True)
                selT = io.tile([3, OW], f32, name="selT")
                SC.activation(selT[:], selP[:], AF.Copy)
                tps = psT.tile([P, P], f32, name="tps", tag="tpose")
                nc.tensor.transpose(tps[0:OW, 0:3], selT[:], ident[0:3, 0:3])
                SC.activation(selsb[:], tps[0:OW, 0:3], AF.Copy)

            xa0 = selsb[:, 0:1]
            xa1 = selsb[:, 1:2]
            qaL = selsb[:, 2:3]
            xasq = io.tile([OW, 1], f32, name="xasq")
            tmpa = io.tile([OW, 1], f32, name="tmpa")
            V.tensor_tensor(tmpa[:], xa0, xa0, OP.mult)
            V.tensor_tensor(xasq[:], xa1, xa1, OP.mult)
            V.tensor_tensor(xasq[:], xasq[:], tmpa[:], OP.add)
            alphaQ = io.tile([OW, 3], f32, name="alphaQ")
            V.tensor_copy(alphaQ[:, 0:1], xa0)
            V.tensor_copy(alphaQ[:, 1:2], xa1)
            V.tensor_copy(alphaQ[:, 2:3], xasq[:])

            # ---------- AG1: alpha coords ----------
            ag_in = dram.tile([1, 3 * OW], f32, name="ag_in")
            ag_out = dram.tile([1, 3 * K], f32, name="ag_out",
                               addr_space="Shared")
            nc.sync.dma_start(ag_in[0:1, :], alphaQ[:])
            nc.gpsimd.collective_compute(
                "AllGather", mybir.AluOpType.bypass,
                replica_groups=GRP,
                ins=[ag_in[:]], outs=[ag_out[:]],
            )

            # ---------- early per-object stats -> coef -> AG2 ----------
            count = cntsb[:, 0:1]
            qseg = cntsb[:, 1:2]

            def otile(name):
                return io.tile([OW, 1], f32, name=name)

            has = otile("has")
            V.tensor_scalar(has[:], count, 0.0, None, OP.is_gt)
            rc = otile("rc")
            V.tensor_scalar(rc[:], count, EPS, None, OP.add)
            V.reciprocal(rc[:], rc[:])
            rnc = otile("rnc")
            V.tensor_scalar(rnc[:], count, -1.0, float(N) + EPS,
                            OP.mult, OP.add)
            V.reciprocal(rnc[:], rnc[:])
            coef = otile("coef")
            V.tensor_tensor(coef[:], qaL, rnc[:], OP.mult)
            V.tensor_tensor(coef[:], coef[:], has[:], OP.mult)

            CH = OW + 4
            ag2_in = dram.tile([1, CH], f32, name="ag2_in")
            ag2_out = dram.tile([1, NCORES * CH], f32, name="ag2_out",
                                addr_space="Shared")
            nc.sync.dma_start(ag2_in[0:1, 0:OW], coef[:])

            # extras: [noise*beta, noise, |x|^2, q] -> [1,4] -> AG2 payload
            is_obj = wtile("is_obj")
            V.tensor_scalar(is_obj[:], sb["tidx"][:], 0.0, None, OP.is_ge)
            is_noise = wtile("is_noise")
            V.scalar_tensor_tensor(is_noise[:], is_obj[:], -1.0,
                                   sb["valid"][:], OP.mult, OP.add)
            extras = io.tile([P, 4], f32, name="extras")
            nb_t = wtile("nb_t")
            V.tensor_tensor(nb_t[:], is_noise[:], beta[:], OP.mult)
            V.tensor_reduce(extras[:, 0:1], nb_t[:], AX.X, OP.add)
            V.tensor_reduce(extras[:, 1:2], is_noise[:], AX.X, OP.add)
            V.tensor_reduce(extras[:, 2:3], prep4[:, :, 3], AX.X, OP.add)
            V.tensor_reduce(extras[:, 3:4], q[:], AX.X, OP.add)
            with tc.tile_pool(name="scpp", bufs=1, space="PSUM") as scp:
                sc1P = scp.tile([1, 4], f32, name="sc1P")
                nc.tensor.matmul(sc1P[:], onescol[:], extras[:],
                                 start=True, stop=True)
                sc1 = io.tile([1, 4], f32, name="sc1")
                SC.activation(sc1[:], sc1P[:], AF.Copy)
            nc.sync.dma_start(ag2_in[0:1, OW:CH], sc1[0:1, :])
            nc.gpsimd.collective_compute(
                "AllGather", mybir.AluOpType.bypass,
                replica_groups=GRP,
                ins=[ag2_in[:]], outs=[ag2_out[:]],
            )

            # ---------- rhsD2ext: [global 512 | own 64] in bf16 ----------
            rhsD2f = io.tile([3, K], f32, name="rhsD2f")
            nc.sync.dma_start(
                rhsD2f[:],
                ag_out[0:1, :].rearrange("o (k r) -> (o r) k", r=3))
            ownT = io.tile([3, OW], f32, name="ownT")
            for r in range(3):
                nc.sync.dma_start(ownT[r:r + 1, :], alphaQ[:, r:r + 1])
            KE = K + OW
            rhsD2 = io.tile([4, KE], bf16, name="rhsD2")
            V.memset(rhsD2[:], 1.0)
            V.tensor_copy(rhsD2[0:3, 0:K], rhsD2f[:])
            V.tensor_copy(rhsD2[0:3, K:KE], ownT[:])

            # ---------- per-hit prep (payload etc) ----------
            wr = wtile("wr")
            SC.activation(wr[:], sb["tE"][:], AF.Identity,
                          bias=cbias(-0.5 / 9.5), scale=1.0 / 9.5)
            ew = wtile("ew")
            V.tensor_scalar(ew[:], wr[:], 1.0, 0.0, OP.min, OP.max)
            pw = wtile("pw")
            V.tensor_tensor(pw[:], beta[:], ew[:], OP.mult)
            V.tensor_tensor(pw[:], pw[:], is_obj[:], OP.mult)

            ediff_r = wtile("ediff_r")
            V.tensor_tensor(ediff_r[:], sb["tE"][:], sb["pE"][:], OP.subtract)
            ediff = wtile("ediff")
            SC.activation(ediff[:], ediff_r[:], AF.Abs)
            ed2 = wtile("ed2")
            V.tensor_tensor(ed2[:], ediff[:], ediff[:], OP.mult)
            ed001 = wtile("ed001")
            SC.activation(ed001[:], ediff[:], AF.Copy, scale=0.001)

            dpos = wtile("dpos", [P, T, 2])
            V.tensor_tensor(dpos[:], sb["tpos"][:], sb["ppos"][:], OP.subtract)
            V.tensor_tensor(dpos[:], dpos[:], dpos[:], OP.mult)
            d2p = wtile("d2p")
            V.tensor_tensor(d2p[:], dpos[:, :, 0], dpos[:, :, 1], OP.add)

            dtm = wtile("dtm")
            V.tensor_tensor(dtm[:], sb["ttime"][:], sb["ptime"][:], OP.subtract)
            adt = wtile("adt")
            SC.activation(adt[:], dtm[:], AF.Abs)
            dt2 = wtile("dt2")
            V.tensor_tensor(dt2[:], dtm[:], dtm[:], OP.mult)
            lint = wtile("lint")
            SC.activation(lint[:], adt[:], AF.Identity, bias=cbias(-4.0),
                          scale=4.0)
            ltt = wtile("ltt", dtype=u8)
            V.tensor_scalar(ltt[:], adt[:], 2.0, None, OP.is_lt)
            ht = wtile("ht")
            V.select(ht[:], ltt[:], dt2[:], lint[:])
            yt = wtile("yt")
            SC.activation(yt[:], ht[:], AF.Copy, scale=1.0 / 6.0)

            pid2 = wtile("pid2", [P, T, 6])
            V.tensor_tensor(pid2[:], sb["pid"][:], sb["pid"][:], OP.mult)
            cred = wtile("cred")
            V.tensor_reduce(cred[:], pid2[:], AX.X, OP.add)

            ex = wtile("ex")
            SC.activation(ex[:], ed2[:], AF.Exp, scale=-0.1)
            xp = wtile("xp")
            SC.activation(xp[:], d2p[:], AF.Sqrt, bias=cbias(0.01), scale=0.01)

            ye = wtile("ye")
            V.tensor_tensor(ye[:], ex[:], ed001[:], OP.add)
            lnye = wtile("lnye")
            SC.activation(lnye[:], ye[:], AF.Ln, bias=cbias(1.0))
            gte = wtile("gte", dtype=u8)
            V.tensor_scalar(gte[:], ye[:], 1.0, None, OP.is_gt)
            esc = wtile("esc")
            V.select(esc[:], gte[:], lnye[:], ye[:])

            xp2 = wtile("xp2")
            V.tensor_tensor(xp2[:], xp[:], xp[:], OP.mult)
            linp = wtile("linp")
            SC.activation(linp[:], xp[:], AF.Identity, bias=cbias(-100.0),
                          scale=20.0)
            ltp = wtile("ltp", dtype=u8)
            V.tensor_scalar(ltp[:], xp[:], 10.0, None, OP.is_lt)
            hp = wtile("hp")
            V.select(hp[:], ltp[:], xp2[:], linp[:])
            yp = wtile("yp")
            SC.activation(yp[:], hp[:], AF.Copy, scale=1.0 / 3.0)
            lnyp = wtile("lnyp")
            SC.activation(lnyp[:], yp[:], AF.Ln, bias=cbias(1.0))
            gtp = wtile("gtp", dtype=u8)
            V.tensor_scalar(gtp[:], yp[:], 1.0, None, OP.is_gt)
            psc = wtile("psc")
            V.select(psc[:], gtp[:], lnyp[:], yp[:])

            lnyt = wtile("lnyt")
            SC.activation(lnyt[:], yt[:], AF.Ln, bias=cbias(1.0))
            gtt = wtile("gtt", dtype=u8)
            V.tensor_scalar(gtt[:], yt[:], 1.0, None, OP.is_gt)
            tsc = wtile("tsc")
            V.select(tsc[:], gtt[:], lnyt[:], yt[:])

            esc10 = wtile("esc10")
            SC.activation(esc10[:], esc[:], AF.Copy, scale=10.0)
            pay = wtile("pay")
            V.scalar_tensor_tensor(pay[:], psc[:], 3.0, esc10[:],
                                   OP.mult, OP.add)
            V.scalar_tensor_tensor(pay[:], tsc[:], 6.0, pay[:],
                                   OP.mult, OP.add)
            V.scalar_tensor_tensor(pay[:], cred[:], 1e-8 / 6.0, pay[:],
                                   OP.mult, OP.add)
            paypw = wtile("paypw")
            V.tensor_tensor(paypw[:], pay[:], pw[:], OP.mult)

            # segment-sum rhs: 5 cols independent of the main loop,
            # accumulated in a mini-loop that hides under the AG1 wait
            rhs_seg = io.tile([P, T, 5], f32, name="rhs_seg")
            V.tensor_tensor(rhs_seg[:, :, 0], qrb[:], prep4[:, :, 3], OP.mult)
            V.tensor_tensor(rhs_seg[:, :, 1], qrb[:], sb["cc"][:, :, 0],
                            OP.mult)
            V.tensor_tensor(rhs_seg[:, :, 2], qrb[:], sb["cc"][:, :, 1],
                            OP.mult)
            V.tensor_tensor(rhs_seg[:, :, 3], pw[:], rbeta[:], OP.mult)
            V.tensor_tensor(rhs_seg[:, :, 4], paypw[:], rbeta[:], OP.mult)

            seg5sb = io.tile([OW, 5], f32, name="seg5sb")
            with tc.tile_pool(name="seg5pp", bufs=1, space="PSUM") as seg5pp:
                seg5P = seg5pp.tile([5, OW], f32, name="seg5P")
                V.memset(seg5P[:], 0.0)
                for t in range(T):
                    nc.tensor.matmul(seg5P[:], rhs_seg[:, t, :],
                                     bmBs[:, t, :],
                                     start=False, stop=(t == T - 1),
                                     skip_group_check=True)
                seg5T = io.tile([5, OW], f32, name="seg5T")
                SC.activation(seg5T[:], seg5P[:], AF.Copy)
                tpg = psT.tile([P, P], f32, name="tpg", tag="tpose")
                nc.tensor.transpose(tpg[0:OW, 0:5], seg5T[:],
                                    ident[0:5, 0:5])
                SC.activation(seg5sb[:], tpg[0:OW, 0:5], AF.Copy)

            # ---------- pre-loop assembly: everything except lr/dot ------
            aqxsq = seg5sb[:, 0:1]
            aqx0 = seg5sb[:, 1:2]
            aqx1 = seg5sb[:, 2:3]
            pwseg = seg5sb[:, 3:4]
            payseg = seg5sb[:, 4:5]

            asm6 = io.tile([OW, 6], f32, name="asm6")
            att = otile("att")
            V.tensor_tensor(att[:], xa0, aqx0, OP.mult)
            tmpb = otile("tmpb")
            V.tensor_tensor(tmpb[:], xa1, aqx1, OP.mult)
            V.tensor_tensor(att[:], att[:], tmpb[:], OP.add)
            V.scalar_tensor_tensor(att[:], att[:], -2.0, aqxsq, OP.mult,
                                   OP.add)
            V.tensor_tensor(tmpb[:], xasq[:], qseg, OP.mult)
            V.tensor_tensor(att[:], att[:], tmpb[:], OP.add)
            V.tensor_tensor(att[:], att[:], qaL, OP.mult)
            V.tensor_tensor(att[:], att[:], rc[:], OP.mult)
            V.tensor_tensor(asm6[:, 0:1], att[:], has[:], OP.mult)
            lb = otile("lb")
            V.tensor_scalar(lb[:], Bmax[:], -1.0, 1.0, OP.mult, OP.add)
            V.tensor_tensor(asm6[:, 1:2], lb[:], has[:], OP.mult)
            lp = otile("lp")
            V.tensor_scalar(lp[:], pwseg, EPS, None, OP.add)
            V.reciprocal(lp[:], lp[:])
            V.tensor_tensor(lp[:], lp[:], payseg, OP.mult)
            V.tensor_tensor(asm6[:, 2:3], lp[:], has[:], OP.mult)
            V.tensor_copy(asm6[:, 3:4], has[:])
            V.tensor_copy(asm6[:, 4:5], coef[:])
            V.tensor_tensor(asm6[:, 5:6], coef[:], qseg, OP.mult)
            fin6 = io.tile([1, 6], f32, name="fin6")
            with tc.tile_pool(name="scp2", bufs=1, space="PSUM") as scp2:
                fin6P = scp2.tile([1, 6], f32, name="fin6P")
                nc.tensor.matmul(fin6P[:], onescol[0:OW, :], asm6[:],
                                 start=True, stop=True)
                SC.activation(fin6[:], fin6P[:], AF.Copy)
            coefT = io.tile([1, OW], f32, name="coefT")
            tpcf = psT.tile([P, P], f32, name="tpcf", tag="tpose")
            nc.tensor.transpose(tpcf[0:1, 0:OW], coef[:], ident[0:OW, 0:OW])
            SC.activation(coefT[:], tpcf[0:1, 0:OW], AF.Copy)

            # close the transpose pool: the main loop needs its PSUM banks
            psTc.__exit__(None, None, None)

            # ---------- main loop: d2 block + rep + self-corr ------------
            qmincol = wtile("qmincol")
            repP = accp.tile([1, K], f32, name="repP")
            V.memset(repP[:], 0.0)
            qminP = accp.tile([1, OW], f32, name="qminP")
            V.memset(qminP[:], 0.0)
            with (
                tc.tile_pool(name="d2pa", bufs=4, space="PSUM") as d2pa,
                tc.tile_pool(name="d2pb", bufs=2, space="PSUM") as d2pb,
                tc.tile_pool(name="sp", bufs=4) as sp,
                tc.tile_pool(name="spw", bufs=4) as spw,
            ):
                for t in range(T):
                    lhs_t = lhsT4[0:4, t, :]
                    d2a = d2pa.tile([P, K], f32, name="d2a")
                    nc.tensor.matmul(d2a[:], lhs_t, rhsD2[:, 0:K],
                                     start=True, stop=True)
                    d2b = d2pb.tile([P, OW], f32, name="d2b")
                    nc.tensor.matmul(d2b[:], lhs_t, rhsD2[:, K:KE],
                                     start=True, stop=True)
                    cl = sp.tile([P, KE], f32, name="cl")
                    V.tensor_scalar(cl[:, 0:K], d2a[:], 1.0, 0.0,
                                    OP.min, OP.max)
                    V.tensor_scalar(cl[:, K:KE], d2b[:], 1.0, 0.0,
                                    OP.min, OP.max)
                    smv = sp.tile([P, KE], bf16, name="smv")
                    SC.activation(smv[:], cl[:], AF.Sqrt, bias=cbias(SQ_BIAS))
                    nc.tensor.matmul(repP[:], qbf[:, t:t + 1], smv[:, 0:K],
                                     start=False, stop=(t == T - 1),
                                     skip_group_check=True)
                    scrW = spw.tile([P, OW], f32, name="scrW")
                    V.scalar_tensor_tensor(scrW[:], bmBs[:, t, :],
                                           qrb2[:, t:t + 1], smv[:, K:KE],
                                           OP.mult, OP.mult,
                                           accum_out=qmincol[:, t:t + 1])
                    nc.tensor.matmul(qminP[:], qmincol[:, t:t + 1],
                                     bmBs[:, t, :],
                                     start=False, stop=(t == T - 1),
                                     skip_group_check=True)

            qminT = io.tile([1, OW], f32, name="qminT")
            SC.activation(qminT[:], qminP[:], AF.Copy)

            # ---------- rep dot + global scalars ----------
            repsb = io.tile([1, K], f32, name="repsb")
            SC.activation(repsb[:], repP[:], AF.Copy)
            ag2sb = io.tile([1, NCORES * CH], f32, name="ag2sb")
            nc.sync.dma_start(ag2sb[:], ag2_out[:])
            cg = io.tile([1, K], f32, name="cg")
            for c in range(NCORES):
                V.tensor_copy(cg[0:1, c * OW:(c + 1) * OW],
                              ag2sb[0:1, c * CH:c * CH + OW])
            exg = io.tile([1, 4], f32, name="exg")
            V.tensor_copy(exg[:], ag2sb[0:1, OW:CH])
            for c in range(1, NCORES):
                V.tensor_tensor(exg[:], exg[:],
                                ag2sb[0:1, c * CH + OW:(c + 1) * CH], OP.add)
            dotv = io.tile([1, K], f32, name="dotv")
            V.tensor_tensor(dotv[:], cg[:], repsb[:], OP.mult)
            dot = io.tile([1, 1], f32, name="dot")
            V.tensor_reduce(dot[:], dotv[:], AX.X, OP.add)

            exB = io.tile([OW, 4], f32, name="exB")
            with tc.tile_pool(name="bcp2", bufs=1, space="PSUM") as bcp2:
                exps = bcp2.tile([OW, 4], f32, name="exps")
                nc.tensor.matmul(exps[:], onesrow[0:1, 0:OW], exg[:],
                                 start=True, stop=True)
                SC.activation(exB[:], exps[:], AF.Copy)
            qsumB = exB[:, 3:4]

            # ---------- per-object assembly ----------
            asm = io.tile([OW, 5], f32, name="asm")
            att = otile("att")
            V.tensor_tensor(att[:], xa0, aqx0, OP.mult)
            tmpb = otile("tmpb")
            V.tensor_tensor(tmpb[:], xa1, aqx1, OP.mult)
            V.tensor_tensor(att[:], att[:], tmpb[:], OP.add)
            V.scalar_tensor_tensor(att[:], att[:], -2.0, aqxsq, OP.mult,
                                   OP.add)
            V.tensor_tensor(tmpb[:], xasq[:], qseg, OP.mult)
            V.tensor_tensor(att[:], att[:], tmpb[:], OP.add)
            V.tensor_tensor(att[:], att[:], qaL, OP.mult)
            V.tensor_tensor(att[:], att[:], rc[:], OP.mult)
            V.tensor_tensor(asm[:, 0:1], att[:], has[:], OP.mult)

            lr = otile("lr")
            V.tensor_tensor(lr[:], qsumB, qseg, OP.subtract)
            V.tensor_tensor(lr[:], lr[:], qmin, OP.add)
            V.tensor_tensor(asm[:, 1:2], lr[:], coef[:], OP.mult)

            lb = otile("lb")
            V.tensor_scalar(lb[:], Bmax[:], -1.0, 1.0, OP.mult, OP.add)
            V.tensor_tensor(asm[:, 2:3], lb[:], has[:], OP.mult)

            lp = otile("lp")
            V.tensor_scalar(lp[:], pwseg, EPS, None, OP.add)
            V.reciprocal(lp[:], lp[:])
            V.tensor_tensor(lp[:], lp[:], payseg, OP.mult)
            V.tensor_tensor(asm[:, 3:4], lp[:], has[:], OP.mult)

            V.tensor_copy(asm[:, 4:5], has[:])

            with tc.tile_pool(name="scp2", bufs=1, space="PSUM") as scp2:
                finP = scp2.tile([1, 5], f32, name="finP")
                nc.tensor.matmul(finP[:], onescol[0:OW, :], asm[:],
                                 start=True, stop=True)
                fin = io.tile([1, 8], f32, name="fin")
                V.memset(fin[:], 0.0)
                SC.activation(fin[0:1, 0:5], finP[:], AF.Copy)
            V.tensor_copy(fin[0:1, 5:6], dot[:])

            # ---------- final AllGather + local sum ----------
            ar_in = dram.tile([1, 8], f32, name="ar_in")
            ar_out = dram.tile([1, 64], f32, name="ar_out",
                               addr_space="Shared")
            nc.sync.dma_start(ar_in[0:1, :], fin[:])
            nc.gpsimd.collective_compute(
                "AllGather", mybir.AluOpType.bypass,
                replica_groups=GRP,
                ins=[ar_in[:]], outs=[ar_out[:]],
            )
            g8 = io.tile([1, 64], f32, name="g8")
            nc.sync.dma_start(g8[:], ar_out[:])
            g = io.tile([1, 8], f32, name="g")
            V.tensor_copy(g[:], g8[0:1, 0:8])
            for c in range(1, NCORES):
                V.tensor_tensor(g[:], g[:], g8[0:1, c * 8:(c + 1) * 8],
                                OP.add)

            # total = (la + (lr_part - dot) + lb + lp)/n_obj + noise + cc
            s4 = io.tile([1, 1], f32, name="s4")
            V.tensor_reduce(s4[:], g[0:1, 0:4], AX.X, OP.add)
            V.tensor_tensor(s4[:], s4[:], g[0:1, 5:6], OP.subtract)
            nobj = io.tile([1, 1], f32, name="nobj")
            V.tensor_scalar(nobj[:], g[0:1, 4:5], EPS, None, OP.add)
            V.reciprocal(nobj[:], nobj[:])
            tot = io.tile([1, 1], f32, name="tot")
            V.tensor_tensor(tot[:], s4[:], nobj[:], OP.mult)
            nden = io.tile([1, 1], f32, name="nden")
            V.tensor_scalar(nden[:], exg[0:1, 1:2], EPS, None, OP.add)
            V.reciprocal(nden[:], nden[:])
            V.tensor_tensor(nden[:], nden[:], exg[0:1, 0:1], OP.mult)
            V.tensor_tensor(tot[:], tot[:], nden[:], OP.add)
            lcc = io.tile([1, 1], f32, name="lcc")
            SC.activation(lcc[:], exg[0:1, 2:3], AF.Copy,
                          scale=0.001 / (2.0 * N))
            V.tensor_tensor(tot[:], tot[:], lcc[:], OP.add)
            nc.sync.dma_start(out_d.ap(), tot[:])

    nc.compile()
    return nc


def _host_prep(inputs):
    """Object-aligned sharding: core c gets the hits of objects
    [64c, 64c+64); noise hits greedily balance the per-core counts."""
    t_idx = inputs["t_idx"][:, 0].astype(np.int64)
    core_of = t_idx // OW                      # noise (-1) -> -1
    idx_lists = [list(np.nonzero(core_of == c)[0]) for c in range(NCORES)]
    counts = np.array([len(l) for l in idx_lists])
    for i in np.nonzero(t_idx < 0)[0]:
        c = int(np.argmin(counts))
        idx_lists[c].append(int(i))
        counts[c] += 1
    T = int(np.ceil(counts.max() / P))
    SP = T * P

    names = {
        "beta_r": "pred_beta", "cc": "pred_ccoords", "pE": "pred_energy",
        "ppos": "pred_pos", "ptime": "pred_time", "pid": "pred_id",
        "tE": "t_energy", "tpos": "t_pos", "ttime": "t_time",
    }

    def lay(a2):                       # [SP, w] -> [128, T, w]
        w = a2.shape[1]
        r = a2.reshape(T, P, w).transpose(1, 0, 2)
        return np.ascontiguousarray(r.astype(np.float32))

    in_maps = []
    for c in range(NCORES):
        sel = np.array(idx_lists[c], dtype=np.int64)
        n = len(sel)

        def pad(a):
            out = np.zeros((SP, a.shape[1]), np.float32)
            out[:n] = a[sel]
            return out

        tl = np.full((SP, 1), -5.0, np.float32)
        tli = t_idx[sel]
        tl[:n, 0] = np.where(tli >= 0, tli - OW * c, -5).astype(np.float32)
        valid = np.zeros((SP, 1), np.float32)
        valid[:n] = 1.0
        m = {"tidx": lay(tl)[:, :, 0], "valid": lay(valid)[:, :, 0]}
        for kn, vn in names.items():
            a = lay(pad(inputs[vn]))
            m[kn] = a if a.shape[2] > 1 else a[:, :, 0]
        m = {k: np.ascontiguousarray(v) for k, v in m.items()}
        in_maps.append(m)
    return in_maps, T


def _run(inputs, trace=False):
    from concourse import bass_utils
    in_maps, T = _host_prep(inputs)
    if T not in _CACHE:
        _CACHE[T] = _build(T)
    nc = _CACHE[T]
    res = bass_utils.run_bass_kernel_spmd(
        nc, in_maps, core_ids=list(range(NCORES)), trace=trace)
    return res


def kernel(**inputs):
    res = _run(inputs, trace=False)
    val = np.float32(res.results[0]["out"][0, 0])
    return np.array(val, dtype=np.float32)[()]


if __name__ == "__main__":
    d = np.load("/tmp/inputs.npz")
    inp = {k: d[k] for k in d.files}
    print("kernel:", kernel(**inp))


# revision 12
# speedup vs baseline: 1.1158x; 1.1158x over previous
"""Trainium2 Bass kernel for LLFullObjectCondensation loss (N=80000, K=512, C=2).

Object-aligned sharding (8 NeuronCores), v4:
  - Core c owns 64 objects; hits of those objects are assigned to core c.
  - Attraction expanded polynomially into segment sums.
  - The repulsion block gets 64 EXTRA columns duplicating the core's own
    object window (from local alphas, no collective needed) so the
    own-alpha hinge values sit at a static column slice; the rep
    self-correction is extracted per tile on the otherwise-idle GpSimd
    engine and feeds the segment matmul in the same loop.
  - L_rep linearized over per-core rep row-sums: per-object coefficients
    are AllGathered and each core contributes a local dot product.
  - Collectives: barrier + AllGather(768B alphas) + AllGather(272B
    coef+extras) + AllGather(32B partials).
"""
import sys
import numpy as np

for _p in ("/opt/trn_rl_repo", "/root/.axon_site/_ro/trn_rl_repo"):
    if _p not in sys.path:
        sys.path.append(_p)

N = 80000
K = 512
NCORES = 8
OW = K // NCORES         # 64 objects per core
P = 128
EPS = 1e-9
SQ_BIAS = 1e-6           # matches reference's d2+1e-6 inside the hinge sqrt

_CACHE = {}


def _build(T):
    import concourse.bass as bass
    import concourse.bacc as bacc
    import concourse.mybir as mybir
    import concourse.tile as tile
    from concourse import masks

    f32 = mybir.dt.float32
    i32 = mybir.dt.int32
    bf16 = mybir.dt.bfloat16
    AF = mybir.ActivationFunctionType
    OP = mybir.AluOpType
    AX = mybir.AxisListType

    nc = bacc.Bacc("TRN2", target_bir_lowering=False, debug=False,
                   num_devices=NCORES)

    di = {}
    def din(name, shape):
        di[name] = nc.dram_tensor(name, shape, f32, kind="ExternalInput")
        return di[name]

    din("beta_r", [P, T])
    din("cc", [P, T, 2])
    din("pE", [P, T])
    din("ppos", [P, T, 2])
    din("ptime", [P, T])
    din("pid", [P, T, 6])
    din("tE", [P, T])
    din("tpos", [P, T, 2])
    din("ttime", [P, T])
    din("tidx", [P, T])
    din("valid", [P, T])
    out_d = nc.dram_tensor("out", [1, 1], f32, kind="ExternalOutput")
    GRP = [list(range(NCORES))]

    with tile.TileContext(nc) as tc:
        with (
            tc.tile_pool(name="const", bufs=1) as cpool,
            tc.tile_pool(name="io", bufs=1) as io,
            tc.tile_pool(name="dram", bufs=1, space="DRAM") as dram,
            tc.tile_pool(name="acc", bufs=1, space="PSUM") as accp,
        ):
            # transpose pool, closed manually before the main loop so its
            # PSUM banks go to the d2 pipeline
            psTc = tc.tile_pool(name="psT", bufs=2, space="PSUM")
            psT = psTc.__enter__()
            # ---------- constants ----------
            ident = cpool.tile([P, P], f32)
            masks.make_identity(nc, ident[:])
            iotaWI = cpool.tile([P, OW], i32)
            nc.gpsimd.iota(iotaWI[:], pattern=[[1, OW]], base=0,
                           channel_multiplier=0)
            iotaW = cpool.tile([P, OW], f32)
            nc.vector.tensor_copy(iotaW[:], iotaWI[:])
            onescol = cpool.tile([P, 1], f32)
            nc.vector.memset(onescol[:], 1.0)
            onesrow = cpool.tile([1, P], f32)
            nc.vector.memset(onesrow[:], 1.0)

            _cb = {}
            def cbias(val):
                if val not in _cb:
                    ct = cpool.tile([P, 1], f32, name=f"cb{len(_cb)}")
                    nc.vector.memset(ct[:], val)
                    _cb[val] = ct
                return _cb[val][:]

            # ---------- early barrier: absorb NEFF launch skew ----------
            bar_in = dram.tile([1, 8], f32, name="bar_in")
            bar_out = dram.tile([1, 64], f32, name="bar_out",
                                addr_space="Shared")
            barsb = cpool.tile([1, 8], f32)
            nc.vector.memset(barsb[:], 0.0)
            nc.sync.dma_start(bar_in[0:1, :], barsb[:])
            nc.gpsimd.collective_compute(
                "AllGather", mybir.AluOpType.bypass,
                replica_groups=GRP,
                ins=[bar_in[:]], outs=[bar_out[:]],
            )

            # ---------- load inputs ----------
            sb = {}
            for name, h in di.items():
                t_sb = io.tile(list(h.shape), f32, name=f"sb_{name}")
                nc.sync.dma_start(t_sb[:], h.ap())
                sb[name] = t_sb

            V = nc.vector
            SC = nc.scalar
            GP = nc.gpsimd

            def wtile(name, shape=None, dtype=None):
                return io.tile(shape or [P, T], dtype or f32, name=name)
            u8 = mybir.dt.uint8

            # ---------- critical chain: beta -> membership + max ----------
            beta = wtile("beta")
            V.tensor_scalar(beta[:], sb["beta_r"][:], 1e-6, 1.0 - 1e-6,
                            OP.max, OP.min)

            # q chain pieces needed for the early count accumulation
            rbeta = wtile("rbeta")
            V.reciprocal(rbeta[:], beta[:])
            betap1 = wtile("betap1")
            SC.activation(betap1[:], beta[:], AF.Identity, bias=cbias(1.0))
            onem = wtile("onem")
            SC.activation(onem[:], beta[:], AF.Identity, bias=cbias(1.0),
                          scale=-1.0)
            recm = wtile("recm")
            V.reciprocal(recm[:], onem[:])
            ratio = wtile("ratio")
            V.tensor_tensor(ratio[:], betap1[:], recm[:], OP.mult)
            lnr = wtile("lnr")
            SC.activation(lnr[:], ratio[:], AF.Ln)
            halfln = wtile("halfln")
            SC.activation(halfln[:], lnr[:], AF.Copy, scale=0.5)
            q = wtile("q")
            V.tensor_tensor(q[:], halfln[:], halfln[:], OP.mult)
            V.scalar_tensor_tensor(q[:], q[:], 0.1, sb["valid"][:],
                                   OP.add, OP.mult)
            qrb = wtile("qrb")
            V.tensor_tensor(qrb[:], q[:], rbeta[:], OP.mult)
            cnt2 = wtile("cnt2", [P, T, 2])
            V.tensor_tensor(cnt2[:, :, 0], sb["valid"][:], rbeta[:], OP.mult)
            V.tensor_copy(cnt2[:, :, 1], qrb[:])

            bmBs = io.tile([P, T, OW], f32, name="bmBs")
            cntsb = io.tile([OW, 2], f32, name="cntsb")
            with tc.tile_pool(name="cntpp", bufs=1, space="PSUM") as cntpp:
                cntP = cntpp.tile([2, OW], f32, name="cntP")
                V.memset(cntP[:], 0.0)
                for t in range(T):
                    V.tensor_scalar(bmBs[:, t, :], iotaW[:],
                                    sb["tidx"][:, t:t + 1], beta[:, t:t + 1],
                                    OP.is_equal, OP.mult)
                    nc.tensor.matmul(cntP[:], cnt2[:, t, :], bmBs[:, t, :],
                                     start=False, stop=(t == T - 1),
                                     skip_group_check=True)
                cntT = io.tile([2, OW], f32, name="cntT")
                SC.activation(cntT[:], cntP[:], AF.Copy)
                tpc = psT.tile([P, P], f32, name="tpc", tag="tpose")
                nc.tensor.transpose(tpc[0:OW, 0:2], cntT[:], ident[0:2, 0:2])
                SC.activation(cntsb[:], tpc[0:OW, 0:2], AF.Copy)

            # pairwise max-reduction tree over tiles (breaks the serial
            # running-max dependency chain)
            mtA = io.tile([P, (T + 1) // 2, OW], f32, name="mtA")
            mtB = io.tile([P, (T + 3) // 4, OW], f32, name="mtB")
            h = T // 2
            V.tensor_tensor(mtA[:, 0:h, :], bmBs[:, 0:2 * h:2, :],
                            bmBs[:, 1:2 * h:2, :], OP.max)
            if T % 2:
                V.tensor_copy(mtA[:, h:h + 1, :], bmBs[:, T - 1:T, :])
                h += 1
            cur, nxt = mtA, mtB
            while h > 1:
                h2 = h // 2
                V.tensor_tensor(nxt[:, 0:h2, :], cur[:, 0:2 * h2:2, :],
                                cur[:, 1:2 * h2:2, :], OP.max)
                if h % 2:
                    V.tensor_copy(nxt[:, h2:h2 + 1, :],
                                  cur[:, h - 1:h, :])
                    h2 += 1
                cur, nxt = nxt, cur
                h = h2
            runmax = cur[:, 0, :]

            Bmax = io.tile([OW, 1], f32, name="Bmax")
            tpm = psT.tile([P, P], f32, name="tpm", tag="tpose")
            nc.tensor.transpose(tpm[0:OW, :], runmax, ident[:])
            V.reduce_max(Bmax[:], tpm[0:OW, :], axis=AX.X)
            BmaxF = io.tile([1, OW], f32, name="BmaxF")
            tpf = psT.tile([P, P], f32, name="tpf", tag="tpose")
            nc.tensor.transpose(tpf[0:1, 0:OW], Bmax[:], ident[0:OW, 0:OW])
            SC.activation(BmaxF[:], tpf[0:1, 0:OW], AF.Copy)
            BlocB = io.tile([P, OW], f32, name="BlocB")
            with tc.tile_pool(name="bcp", bufs=1, space="PSUM") as bcp:
                bps = bcp.tile([P, OW], f32, name="bps")
                nc.tensor.matmul(bps[:], onesrow[:], BmaxF[:],
                                 start=True, stop=True)
                SC.activation(BlocB[:], bps[:], AF.Copy)

            # ---------- remaining q-derived quantities ----------
            qrb2 = wtile("qrb2")
            V.tensor_tensor(qrb2[:], qrb[:], rbeta[:], OP.mult)
            qbf = wtile("qbf", dtype=bf16)
            SC.activation(qbf[:], q[:], AF.Copy)

            # d2-matmul lhsT quantities + transposes
            prep4 = wtile("prep4", [P, T, 4])
            SC.activation(prep4[:, :, 0:2], sb["cc"][:], AF.Copy, scale=-2.0)
            V.memset(prep4[:, :, 2], 1.0)
            ccsq = wtile("ccsq", [P, T, 2])
            V.tensor_tensor(ccsq[:], sb["cc"][:], sb["cc"][:], OP.mult)
            V.tensor_tensor(prep4[:, :, 3], ccsq[:, :, 0], ccsq[:, :, 1],
                            OP.add)
            lhsT4 = io.tile([4, T, P], bf16, name="lhsT4")
            for r in range(4):
                tp = psT.tile([P, P], f32, name="tpose3", tag="tpose")
                nc.tensor.transpose(tp[0:T, :], prep4[:, :, r], ident[:])
                stage = io.tile([T, P], bf16, name=f"tstage{r}")
                SC.activation(stage[:], tp[0:T, :], AF.Copy)
                nc.sync.dma_start(lhsT4[r:r + 1, :, :], stage[:])

            # selection rhs: [x0, x1, q]
            sel3 = wtile("sel3", [P, T, 3])
            SC.activation(sel3[:, :, 0:2], sb["cc"][:], AF.Copy)
            V.tensor_copy(sel3[:, :, 2], q[:])

            # ---------- loopB: alpha selection (core-local) ----------
            selsb = io.tile([OW, 3], f32, name="selsb")
            with (
                tc.tile_pool(name="selpp", bufs=1, space="PSUM") as selpp,
                tc.tile_pool(name="isp", bufs=3) as isp,
            ):
                selP = selpp.tile([3, OW], f32, name="selP")
                V.memset(selP[:], 0.0)
                for t in range(T):
                    Isel = isp.tile([P, OW], f32, name="Isel")
                    V.tensor_tensor(Isel[:], bmBs[:, t, :], BlocB[:],
                                    OP.is_equal)
                    nc.tensor.matmul(selP[:], sel3[:, t, :], Isel[:],
                                     start=False, stop=(t == T - 1),
                                     skip_group_check=True)
                selT = io.tile([3, OW], f32, name="selT")
                SC.activation(selT[:], selP[:], AF.Copy)
                tps = psT.tile([P, P], f32, name="tps", tag="tpose")
                nc.tensor.transpose(tps[0:OW, 0:3], selT[:], ident[0:3, 0:3])
                SC.activation(selsb[:], tps[0:OW, 0:3], AF.Copy)

            xa0 = selsb[:, 0:1]
            xa1 = selsb[:, 1:2]
            qaL = selsb[:, 2:3]
            xasq = io.tile([OW, 1], f32, name="xasq")
            tmpa = io.tile([OW, 1], f32, name="tmpa")
            V.tensor_tensor(tmpa[:], xa0, xa0, OP.mult)
            V.tensor_tensor(xasq[:], xa1, xa1, OP.mult)
            V.tensor_tensor(xasq[:], xasq[:], tmpa[:], OP.add)
            alphaQ = io.tile([OW, 3], f32, name="alphaQ")
            V.tensor_copy(alphaQ[:, 0:1], xa0)
            V.tensor_copy(alphaQ[:, 1:2], xa1)
            V.tensor_copy(alphaQ[:, 2:3], xasq[:])

            # ---------- AG1: alpha coords ----------
            ag_in = dram.tile([1, 3 * OW], f32, name="ag_in")
            ag_out = dram.tile([1, 3 * K], f32, name="ag_out",
                               addr_space="Shared")
            nc.sync.dma_start(ag_in[0:1, :], alphaQ[:])
            nc.gpsimd.collective_compute(
                "AllGather", mybir.AluOpType.bypass,
                replica_groups=GRP,
                ins=[ag_in[:]], outs=[ag_out[:]],
            )

            # ---------- early per-object stats -> coef -> AG2 ----------
            count = cntsb[:, 0:1]
            qseg = cntsb[:, 1:2]

            def otile(name):
                return io.tile([OW, 1], f32, name=name)

            has = otile("has")
            V.tensor_scalar(has[:], count, 0.0, None, OP.is_gt)
            rc = otile("rc")
            V.tensor_scalar(rc[:], count, EPS, None, OP.add)
            V.reciprocal(rc[:], rc[:])
            rnc = otile("rnc")
            V.tensor_scalar(rnc[:], count, -1.0, float(N) + EPS,
                            OP.mult, OP.add)
            V.reciprocal(rnc[:], rnc[:])
            coef = otile("coef")
            V.tensor_tensor(coef[:], qaL, rnc[:], OP.mult)
            V.tensor_tensor(coef[:], coef[:], has[:], OP.mult)

            CH = OW + 4
            ag2_in = dram.tile([1, CH], f32, name="ag2_in")
            ag2_out = dram.tile([1, NCORES * CH], f32, name="ag2_out",
                                addr_space="Shared")
            nc.sync.dma_start(ag2_in[0:1, 0:OW], coef[:])

            # extras: [noise*beta, noise, |x|^2, q] -> [1,4] -> AG2 payload
            is_obj = wtile("is_obj")
            V.tensor_scalar(is_obj[:], sb["tidx"][:], 0.0, None, OP.is_ge)
            is_noise = wtile("is_noise")
            V.scalar_tensor_tensor(is_noise[:], is_obj[:], -1.0,
                                   sb["valid"][:], OP.mult, OP.add)
            extras = io.tile([P, 4], f32, name="extras")
            nb_t = wtile("nb_t")
            V.tensor_tensor(nb_t[:], is_noise[:], beta[:], OP.mult)
            V.tensor_reduce(extras[:, 0:1], nb_t[:], AX.X, OP.add)
            V.tensor_reduce(extras[:, 1:2], is_noise[:], AX.X, OP.add)
            V.tensor_reduce(extras[:, 2:3], prep4[:, :, 3], AX.X, OP.add)
            V.tensor_reduce(extras[:, 3:4], q[:], AX.X, OP.add)
            with tc.tile_pool(name="scpp", bufs=1, space="PSUM") as scp:
                sc1P = scp.tile([1, 4], f32, name="sc1P")
                nc.tensor.matmul(sc1P[:], onescol[:], extras[:],
                                 start=True, stop=True)
                sc1 = io.tile([1, 4], f32, name="sc1")
                SC.activation(sc1[:], sc1P[:], AF.Copy)
            nc.sync.dma_start(ag2_in[0:1, OW:CH], sc1[0:1, :])
            nc.gpsimd.collective_compute(
                "AllGather", mybir.AluOpType.bypass,
                replica_groups=GRP,
                ins=[ag2_in[:]], outs=[ag2_out[:]],
            )

            # ---------- rhsD2ext: [global 512 | own 64] in bf16 ----------
            rhsD2f = io.tile([3, K], f32, name="rhsD2f")
            nc.sync.dma_start(
                rhsD2f[:],
                ag_out[0:1, :].rearrange("o (k r) -> (o r) k", r=3))
            ownT = io.tile([3, OW], f32, name="ownT")
            for r in range(3):
                nc.sync.dma_start(ownT[r:r + 1, :], alphaQ[:, r:r + 1])
            KE = K + OW
            rhsD2 = io.tile([4, KE], bf16, name="rhsD2")
            V.memset(rhsD2[:], 1.0)
            V.tensor_copy(rhsD2[0:3, 0:K], rhsD2f[:])
            V.tensor_copy(rhsD2[0:3, K:KE], ownT[:])

            # ---------- per-hit prep (payload etc) ----------
            wr = wtile("wr")
            SC.activation(wr[:], sb["tE"][:], AF.Identity,
                          bias=cbias(-0.5 / 9.5), scale=1.0 / 9.5)
            ew = wtile("ew")
            V.tensor_scalar(ew[:], wr[:], 1.0, 0.0, OP.min, OP.max)
            pw = wtile("pw")
            V.tensor_tensor(pw[:], beta[:], ew[:], OP.mult)
            V.tensor_tensor(pw[:], pw[:], is_obj[:], OP.mult)

            ediff_r = wtile("ediff_r")
            V.tensor_tensor(ediff_r[:], sb["tE"][:], sb["pE"][:], OP.subtract)
            ediff = wtile("ediff")
            SC.activation(ediff[:], ediff_r[:], AF.Abs)
            ed2 = wtile("ed2")
            V.tensor_tensor(ed2[:], ediff[:], ediff[:], OP.mult)
            ed001 = wtile("ed001")
            SC.activation(ed001[:], ediff[:], AF.Copy, scale=0.001)

            dpos = wtile("dpos", [P, T, 2])
            V.tensor_tensor(dpos[:], sb["tpos"][:], sb["ppos"][:], OP.subtract)
            V.tensor_tensor(dpos[:], dpos[:], dpos[:], OP.mult)
            d2p = wtile("d2p")
            V.tensor_tensor(d2p[:], dpos[:, :, 0], dpos[:, :, 1], OP.add)

            dtm = wtile("dtm")
            V.tensor_tensor(dtm[:], sb["ttime"][:], sb["ptime"][:], OP.subtract)
            adt = wtile("adt")
            SC.activation(adt[:], dtm[:], AF.Abs)
            dt2 = wtile("dt2")
            V.tensor_tensor(dt2[:], dtm[:], dtm[:], OP.mult)
            lint = wtile("lint")
            SC.activation(lint[:], adt[:], AF.Identity, bias=cbias(-4.0),
                          scale=4.0)
            ltt = wtile("ltt", dtype=u8)
            V.tensor_scalar(ltt[:], adt[:], 2.0, None, OP.is_lt)
            ht = wtile("ht")
            V.select(ht[:], ltt[:], dt2[:], lint[:])
            yt = wtile("yt")
            SC.activation(yt[:], ht[:], AF.Copy, scale=1.0 / 6.0)

            pid2 = wtile("pid2", [P, T, 6])
            V.tensor_tensor(pid2[:], sb["pid"][:], sb["pid"][:], OP.mult)
            cred = wtile("cred")
            V.tensor_reduce(cred[:], pid2[:], AX.X, OP.add)

            ex = wtile("ex")
            SC.activation(ex[:], ed2[:], AF.Exp, scale=-0.1)
            xp = wtile("xp")
            SC.activation(xp[:], d2p[:], AF.Sqrt, bias=cbias(0.01), scale=0.01)

            ye = wtile("ye")
            V.tensor_tensor(ye[:], ex[:], ed001[:], OP.add)
            lnye = wtile("lnye")
            SC.activation(lnye[:], ye[:], AF.Ln, bias=cbias(1.0))
            gte = wtile("gte", dtype=u8)
            V.tensor_scalar(gte[:], ye[:], 1.0, None, OP.is_gt)
            esc = wtile("esc")
            V.select(esc[:], gte[:], lnye[:], ye[:])

            xp2 = wtile("xp2")
            V.tensor_tensor(xp2[:], xp[:], xp[:], OP.mult)
            linp = wtile("linp")
            SC.activation(linp[:], xp[:], AF.Identity, bias=cbias(-100.0),
                          scale=20.0)
            ltp = wtile("ltp", dtype=u8)
            V.tensor_scalar(ltp[:], xp[:], 10.0, None, OP.is_lt)
            hp = wtile("hp")
            V.select(hp[:], ltp[:], xp2[:], linp[:])
            yp = wtile("yp")
            SC.activation(yp[:], hp[:], AF.Copy, scale=1.0 / 3.0)
            lnyp = wtile("lnyp")
            SC.activation(lnyp[:], yp[:], AF.Ln, bias=cbias(1.0))
            gtp = wtile("gtp", dtype=u8)
            V.tensor_scalar(gtp[:], yp[:], 1.0, None, OP.is_gt)
            psc = wtile("psc")
            V.select(psc[:], gtp[:], lnyp[:], yp[:])

            lnyt = wtile("lnyt")
            SC.activation(lnyt[:], yt[:], AF.Ln, bias=cbias(1.0))
            gtt = wtile("gtt", dtype=u8)
            V.tensor_scalar(gtt[:], yt[:], 1.0, None, OP.is_gt)
            tsc = wtile("tsc")
            V.select(tsc[:], gtt[:], lnyt[:], yt[:])

            esc10 = wtile("esc10")
            SC.activation(esc10[:], esc[:], AF.Copy, scale=10.0)
            pay = wtile("pay")
            V.scalar_tensor_tensor(pay[:], psc[:], 3.0, esc10[:],
                                   OP.mult, OP.add)
            V.scalar_tensor_tensor(pay[:], tsc[:], 6.0, pay[:],
                                   OP.mult, OP.add)
            V.scalar_tensor_tensor(pay[:], cred[:], 1e-8 / 6.0, pay[:],
                                   OP.mult, OP.add)
            paypw = wtile("paypw")
            V.tensor_tensor(paypw[:], pay[:], pw[:], OP.mult)

            # segment-sum rhs: 5 cols independent of the main loop,
            # accumulated in a mini-loop that hides under the AG1 wait
            rhs_seg = io.tile([P, T, 5], f32, name="rhs_seg")
            V.tensor_tensor(rhs_seg[:, :, 0], qrb[:], prep4[:, :, 3], OP.mult)
            V.tensor_tensor(rhs_seg[:, :, 1], qrb[:], sb["cc"][:, :, 0],
                            OP.mult)
            V.tensor_tensor(rhs_seg[:, :, 2], qrb[:], sb["cc"][:, :, 1],
                            OP.mult)
            V.tensor_tensor(rhs_seg[:, :, 3], pw[:], rbeta[:], OP.mult)
            V.tensor_tensor(rhs_seg[:, :, 4], paypw[:], rbeta[:], OP.mult)

            seg5sb = io.tile([OW, 5], f32, name="seg5sb")
            with tc.tile_pool(name="seg5pp", bufs=1, space="PSUM") as seg5pp:
                seg5P = seg5pp.tile([5, OW], f32, name="seg5P")
                V.memset(seg5P[:], 0.0)
                for t in range(T):
                    nc.tensor.matmul(seg5P[:], rhs_seg[:, t, :],
                                     bmBs[:, t, :],
                                     start=False, stop=(t == T - 1),
                                     skip_group_check=True)
                seg5T = io.tile([5, OW], f32, name="seg5T")
                SC.activation(seg5T[:], seg5P[:], AF.Copy)
                tpg = psT.tile([P, P], f32, name="tpg", tag="tpose")
                nc.tensor.transpose(tpg[0:OW, 0:5], seg5T[:],
                                    ident[0:5, 0:5])
                SC.activation(seg5sb[:], tpg[0:OW, 0:5], AF.Copy)

            # ---------- pre-loop assembly: everything except lr/dot ------
            aqxsq = seg5sb[:, 0:1]
            aqx0 = seg5sb[:, 1:2]
            aqx1 = seg5sb[:, 2:3]
            pwseg = seg5sb[:, 3:4]
            payseg = seg5sb[:, 4:5]

            asm6 = io.tile([OW, 6], f32, name="asm6")
            att = otile("att")
            V.tensor_tensor(att[:], xa0, aqx0, OP.mult)
            tmpb = otile("tmpb")
            V.tensor_tensor(tmpb[:], xa1, aqx1, OP.mult)
            V.tensor_tensor(att[:], att[:], tmpb[:], OP.add)
            V.scalar_tensor_tensor(att[:], att[:], -2.0, aqxsq, OP.mult,
                                   OP.add)
            V.tensor_tensor(tmpb[:], xasq[:], qseg, OP.mult)
            V.tensor_tensor(att[:], att[:], tmpb[:], OP.add)
            V.tensor_tensor(att[:], att[:], qaL, OP.mult)
            V.tensor_tensor(att[:], att[:], rc[:], OP.mult)
            V.tensor_tensor(asm6[:, 0:1], att[:], has[:], OP.mult)
            lb = otile("lb")
            V.tensor_scalar(lb[:], Bmax[:], -1.0, 1.0, OP.mult, OP.add)
            V.tensor_tensor(asm6[:, 1:2], lb[:], has[:], OP.mult)
            lp = otile("lp")
            V.tensor_scalar(lp[:], pwseg, EPS, None, OP.add)
            V.reciprocal(lp[:], lp[:])
            V.tensor_tensor(lp[:], lp[:], payseg, OP.mult)
            V.tensor_tensor(asm6[:, 2:3], lp[:], has[:], OP.mult)
            V.tensor_copy(asm6[:, 3:4], has[:])
            V.tensor_copy(asm6[:, 4:5], coef[:])
            V.tensor_tensor(asm6[:, 5:6], coef[:], qseg, OP.mult)
            fin6 = io.tile([1, 6], f32, name="fin6")
            with tc.tile_pool(name="scp2", bufs=1, space="PSUM") as scp2:
                fin6P = scp2.tile([1, 6], f32, name="fin6P")
                nc.tensor.matmul(fin6P[:], onescol[0:OW, :], asm6[:],
                                 start=True, stop=True)
                SC.activation(fin6[:], fin6P[:], AF.Copy)
            coefT = io.tile([1, OW], f32, name="coefT")
            tpcf = psT.tile([P, P], f32, name="tpcf", tag="tpose")
            nc.tensor.transpose(tpcf[0:1, 0:OW], coef[:], ident[0:OW, 0:OW])
            SC.activation(coefT[:], tpcf[0:1, 0:OW], AF.Copy)

            # close the transpose pool: the main loop needs its PSUM banks
            psTc.__exit__(None, None, None)

            # ---------- main loop: d2 block + rep + self-corr ------------
            qmincol = wtile("qmincol")
            repP = accp.tile([1, K], f32, name="repP")
            V.memset(repP[:], 0.0)
            qminP = accp.tile([1, OW], f32, name="qminP")
            V.memset(qminP[:], 0.0)
            with (
                tc.tile_pool(name="d2pa", bufs=4, space="PSUM") as d2pa,
                tc.tile_pool(name="d2pb", bufs=2, space="PSUM") as d2pb,
                tc.tile_pool(name="sp", bufs=6) as sp,
                tc.tile_pool(name="spw", bufs=4) as spw,
            ):
                for t in range(T):
                    lhs_t = lhsT4[0:4, t, :]
                    d2a = d2pa.tile([P, K], f32, name="d2a")
                    nc.tensor.matmul(d2a[:], lhs_t, rhsD2[:, 0:K],
                                     start=True, stop=True)
                    d2b = d2pb.tile([P, OW], f32, name="d2b")
                    nc.tensor.matmul(d2b[:], lhs_t, rhsD2[:, K:KE],
                                     start=True, stop=True)
                    cl = sp.tile([P, KE], f32, name="cl")
                    V.tensor_scalar(cl[:, 0:K], d2a[:], 1.0, 0.0,
                                    OP.min, OP.max)
                    V.tensor_scalar(cl[:, K:KE], d2b[:], 1.0, 0.0,
                                    OP.min, OP.max)
                    smv = sp.tile([P, KE], bf16, name="smv")
                    SC.activation(smv[:], cl[:], AF.Sqrt, bias=cbias(SQ_BIAS))
                    nc.tensor.matmul(repP[:], qbf[:, t:t + 1], smv[:, 0:K],
                                     start=False, stop=(t == T - 1),
                                     skip_group_check=True)
                    scrW = spw.tile([P, OW], f32, name="scrW")
                    V.scalar_tensor_tensor(scrW[:], bmBs[:, t, :],
                                           qrb2[:, t:t + 1], smv[:, K:KE],
                                           OP.mult, OP.mult,
                                           accum_out=qmincol[:, t:t + 1])
                    nc.tensor.matmul(qminP[:], qmincol[:, t:t + 1],
                                     bmBs[:, t, :],
                                     start=False, stop=(t == T - 1),
                                     skip_group_check=True)

            qminT = io.tile([1, OW], f32, name="qminT")
            SC.activation(qminT[:], qminP[:], AF.Copy)

            # ---------- rep dot + global scalars ----------
            repsb = io.tile([1, K], f32, name="repsb")
            SC.activation(repsb[:], repP[:], AF.Copy)
            ag2sb = io.tile([1, NCORES * CH], f32, name="ag2sb")
            nc.sync.dma_start(ag2sb[:], ag2_out[:])
            cg = io.tile([1, K], f32, name="cg")
            for c in range(NCORES):
                V.tensor_copy(cg[0:1, c * OW:(c + 1) * OW],
                              ag2sb[0:1, c * CH:c * CH + OW])
            exg = io.tile([1, 4], f32, name="exg")
            V.tensor_copy(exg[:], ag2sb[0:1, OW:CH])
            for c in range(1, NCORES):
                V.tensor_tensor(exg[:], exg[:],
                                ag2sb[0:1, c * CH + OW:(c + 1) * CH], OP.add)
            dotv = io.tile([1, K], f32, name="dotv")
            V.tensor_tensor(dotv[:], cg[:], repsb[:], OP.mult)
            dot = io.tile([1, 1], f32, name="dot")
            V.tensor_reduce(dot[:], dotv[:], AX.X, OP.add)

            exB = io.tile([OW, 4], f32, name="exB")
            with tc.tile_pool(name="bcp2", bufs=1, space="PSUM") as bcp2:
                exps = bcp2.tile([OW, 4], f32, name="exps")
                nc.tensor.matmul(exps[:], onesrow[0:1, 0:OW], exg[:],
                                 start=True, stop=True)
                SC.activation(exB[:], exps[:], AF.Copy)
            qsumB = exB[:, 3:4]

            # ---------- per-object assembly ----------
            asm = io.tile([OW, 5], f32, name="asm")
            att = otile("att")
            V.tensor_tensor(att[:], xa0, aqx0, OP.mult)
            tmpb = otile("tmpb")
            V.tensor_tensor(tmpb[:], xa1, aqx1, OP.mult)
            V.tensor_tensor(att[:], att[:], tmpb[:], OP.add)
            V.scalar_tensor_tensor(att[:], att[:], -2.0, aqxsq, OP.mult,
                                   OP.add)
            V.tensor_tensor(tmpb[:], xasq[:], qseg, OP.mult)
            V.tensor_tensor(att[:], att[:], tmpb[:], OP.add)
            V.tensor_tensor(att[:], att[:], qaL, OP.mult)
            V.tensor_tensor(att[:], att[:], rc[:], OP.mult)
            V.tensor_tensor(asm[:, 0:1], att[:], has[:], OP.mult)

            lr = otile("lr")
            V.tensor_tensor(lr[:], qsumB, qseg, OP.subtract)
            V.tensor_tensor(lr[:], lr[:], qmin, OP.add)
            V.tensor_tensor(asm[:, 1:2], lr[:], coef[:], OP.mult)

            lb = otile("lb")
            V.tensor_scalar(lb[:], Bmax[:], -1.0, 1.0, OP.mult, OP.add)
            V.tensor_tensor(asm[:, 2:3], lb[:], has[:], OP.mult)

            lp = otile("lp")
            V.tensor_scalar(lp[:], pwseg, EPS, None, OP.add)
            V.reciprocal(lp[:], lp[:])
            V.tensor_tensor(lp[:], lp[:], payseg, OP.mult)
            V.tensor_tensor(asm[:, 3:4], lp[:], has[:], OP.mult)

            V.tensor_copy(asm[:, 4:5], has[:])

            with tc.tile_pool(name="scp2", bufs=1, space="PSUM") as scp2:
                finP = scp2.tile([1, 5], f32, name="finP")
                nc.tensor.matmul(finP[:], onescol[0:OW, :], asm[:],
                                 start=True, stop=True)
                fin = io.tile([1, 8], f32, name="fin")
                V.memset(fin[:], 0.0)
                SC.activation(fin[0:1, 0:5], finP[:], AF.Copy)
            V.tensor_copy(fin[0:1, 5:6], dot[:])

            # ---------- final AllGather + local sum ----------
            ar_in = dram.tile([1, 8], f32, name="ar_in")
            ar_out = dram.tile([1, 64], f32, name="ar_out",
                               addr_space="Shared")
            nc.sync.dma_start(ar_in[0:1, :], fin[:])
            nc.gpsimd.collective_compute(
                "AllGather", mybir.AluOpType.bypass,
                replica_groups=GRP,
                ins=[ar_in[:]], outs=[ar_out[:]],
            )
            g8 = io.tile([1, 64], f32, name="g8")
            nc.sync.dma_start(g8[:], ar_out[:])
            g = io.tile([1, 8], f32, name="g")
            V.tensor_copy(g[:], g8[0:1, 0:8])
            for c in range(1, NCORES):
                V.tensor_tensor(g[:], g[:], g8[0:1, c * 8:(c + 1) * 8],
                                OP.add)

            # total = (la + (lr_part - dot) + lb + lp)/n_obj + noise + cc
            s4 = io.tile([1, 1], f32, name="s4")
            V.tensor_reduce(s4[:], g[0:1, 0:4], AX.X, OP.add)
            V.tensor_tensor(s4[:], s4[:], g[0:1, 5:6], OP.subtract)
            nobj = io.tile([1, 1], f32, name="nobj")
            V.tensor_scalar(nobj[:], g[0:1, 4:5], EPS, None, OP.add)
            V.reciprocal(nobj[:], nobj[:])
            tot = io.tile([1, 1], f32, name="tot")
            V.tensor_tensor(tot[:], s4[:], nobj[:], OP.mult)
            nden = io.tile([1, 1], f32, name="nden")
            V.tensor_scalar(nden[:], exg[0:1, 1:2], EPS, None, OP.add)
            V.reciprocal(nden[:], nden[:])
            V.tensor_tensor(nden[:], nden[:], exg[0:1, 0:1], OP.mult)
            V.tensor_tensor(tot[:], tot[:], nden[:], OP.add)
            lcc = io.tile([1, 1], f32, name="lcc")
            SC.activation(lcc[:], exg[0:1, 2:3], AF.Copy,
                          scale=0.001 / (2.0 * N))
            V.tensor_tensor(tot[:], tot[:], lcc[:], OP.add)
            nc.sync.dma_start(out_d.ap(), tot[:])

    nc.compile()
    return nc


def _host_prep(inputs):
    """Object-aligned sharding: core c gets the hits of objects
    [64c, 64c+64); noise hits greedily balance the per-core counts."""
    t_idx = inputs["t_idx"][:, 0].astype(np.int64)
    core_of = t_idx // OW                      # noise (-1) -> -1
    idx_lists = [list(np.nonzero(core_of == c)[0]) for c in range(NCORES)]
    counts = np.array([len(l) for l in idx_lists])
    for i in np.nonzero(t_idx < 0)[0]:
        c = int(np.argmin(counts))
        idx_lists[c].append(int(i))
        counts[c] += 1
    T = int(np.ceil(counts.max() / P))
    SP = T * P

    names = {
        "beta_r": "pred_beta", "cc": "pred_ccoords", "pE": "pred_energy",
        "ppos": "pred_pos", "ptime": "pred_time", "pid": "pred_id",
        "tE": "t_energy", "tpos": "t_pos", "ttime": "t_time",
    }

    def lay(a2):                       # [SP, w] -> [128, T, w]
        w = a2.shape[1]
        r = a2.reshape(T, P, w).transpose(1, 0, 2)
        return np.ascontiguousarray(r.astype(np.float32))

    in_maps = []
    for c in range(NCORES):
        sel = np.array(idx_lists[c], dtype=np.int64)
        n = len(sel)

        def pad(a):
            out = np.zeros((SP, a.shape[1]), np.float32)
            out[:n] = a[sel]
            return out

        tl = np.full((SP, 1), -5.0, np.float32)
        tli = t_idx[sel]
        tl[:n, 0] = np.where(tli >= 0, tli - OW * c, -5).astype(np.float32)
        valid = np.zeros((SP, 1), np.float32)
        valid[:n] = 1.0
        m = {"tidx": lay(tl)[:, :, 0], "valid": lay(valid)[:, :, 0]}
        for kn, vn in names.items():
            a = lay(pad(inputs[vn]))
            m[kn] = a if a.shape[2] > 1 else a[:, :, 0]
        m = {k: np.ascontiguousarray(v) for k, v in m.items()}
        in_maps.append(m)
    return in_maps, T


def _run(inputs, trace=False):
    from concourse import bass_utils
    in_maps, T = _host_prep(inputs)
    if T not in _CACHE:
        _CACHE[T] = _build(T)
    nc = _CACHE[T]
    res = bass_utils.run_bass_kernel_spmd(
        nc, in_maps, core_ids=list(range(NCORES)), trace=trace)
    return res


def kernel(**inputs):
    res = _run(inputs, trace=False)
    val = np.float32(res.results[0]["out"][0, 0])
    return np.array(val, dtype=np.float32)[()]


if __name__ == "__main__":
    d = np.load("/tmp/inputs.npz")
    inp = {k: d[k] for k in d.files}
    print("kernel:", kernel(**inp))
